# revision 67
# baseline (speedup 1.0000x reference)
"""MoE FFN (E=8 experts, top-2) Trainium2 Bass kernel.

Strategy: data-parallel over tokens across 8 NeuronCores, expert weights
replicated.  Each core processes TC = T/8 = 2048 tokens through all 8
experts densely; the top-2 gate (computed exactly in fp32 on-device)
zeroes the non-selected experts, which reproduces the reference MoE math
exactly.

Layout trick: the host feeds x transposed ([D, TC] per core) so the
contraction dim D lands on SBUF partitions; weights are host-pre-transposed
the same way.  All device compute is token-major:
  mm1:  h[t, (e,de)]  = sum_dc xT[dc,t].T @ W1T[dc,(e,de)]
  gate: top-2 over 8 logits via Max8, weights via sigmoid(l1-l2)
  hg   = relu(h) * gate_e   (ACT relu from PSUM + DVE broadcast multiply)
  hgT  = PE transpose (128x128 blocks, bf16)
  mm2:  y[t, d]  = sum_e hgT[de,t].T @ W2T[de,(e),d]
Host does only layout transposes + dtype casts + shard/concat (no model
math).

Software pipelining (PE is the bottleneck engine at ~95% busy):
  - the gate logits + serial top-2 DVE/ACT chain for tile t+1 are issued
    during tile t's matmuls, so the chain latency never stalls the PE;
  - each tile's transposes+mm2 run one tile LATE (during tile t's window
    the PE executes mm1(t), gate-mm(t+1), then tp/mm2(t-1)), so the
    relu->gate-multiply latency is fully hidden;
  - tile 0's x DMA is issued before the expert weights, and w1/w2 chunk
    DMAs are interleaved so early matmuls aren't starved.

Precision: the expert matmuls run in bf16 (x cast on-chip by ACT, W1/W2
cast by the host; PSUM accumulation is fp32), and y ships back as bf16
(halves the output HBM traffic) — measured rel err ~4.3e-3 against the
fp32 reference, vs the 2e-2 gate.  The gate logits run in exact fp32 via
the fp32-tagged view of each x tile, because low-precision logit noise
flips the top-2 selection for near-tie tokens (min observed |l2-l3| gap
is 3.3e-6), which the absmax check would catch.
"""

import os
import sys

import numpy as np

if "/opt/trn_rl_repo" not in sys.path:
    sys.path.insert(0, "/opt/trn_rl_repo")

from contextlib import ExitStack

import concourse.bacc as bacc
import concourse.mybir as mybir
import concourse.tile as tile
from concourse.bass_utils import run_bass_kernel_spmd
from concourse.masks import make_identity

F32 = mybir.dt.float32
F32R = mybir.dt.float32r
BF16 = mybir.dt.bfloat16

B, S, D = 4, 4096, 1024
DE, E = 128, 8
NCORES = 8
T = B * S                 # 16384 tokens
TC = T // NCORES          # 2048 tokens per core
NTT = TC // 128           # 16 token tiles per core
NDC = D // 128            # 8 contraction chunks

_LAST_RESULT = None
_NC_CACHE = {}


def build_moe_nc(with_bias: bool, reps: int = 1):
    # reps > 1 repeats the whole compute pipeline (for timing-slope
    # measurement in test.py); the graded path always uses reps=1.
    nc = bacc.Bacc(None, target_bir_lowering=False)

    xT = nc.declare_dram_parameter("xT", [D, TC], F32R, isOutput=False)
    w1t = nc.declare_dram_parameter("w1t", [D, E * DE], BF16, isOutput=False)
    wgt = nc.declare_dram_parameter("wgt", [D, E], F32, isOutput=False)
    w2t = nc.declare_dram_parameter("w2t", [E * DE, D], BF16, isOutput=False)
    if with_bias:
        b1d = nc.declare_dram_parameter("b1", [1, E, DE], F32, isOutput=False)
        b2d = nc.declare_dram_parameter("b2", [E, D], F32, isOutput=False)
    y = nc.declare_dram_parameter("y", [TC, D], BF16, isOutput=True)

    with tile.TileContext(nc) as tc, ExitStack() as ctx:
        consts = ctx.enter_context(tc.tile_pool(name="consts", bufs=1))
        sbuf = ctx.enter_context(tc.tile_pool(name="sbuf", bufs=2))
        xpool = ctx.enter_context(tc.tile_pool(name="xpool", bufs=3))
        psA = ctx.enter_context(tc.tile_pool(name="psA", bufs=1, space="PSUM"))
        psT = ctx.enter_context(tc.tile_pool(name="psT", bufs=2, space="PSUM"))
        psY = ctx.enter_context(tc.tile_pool(name="psY", bufs=1, space="PSUM"))
        psG = ctx.enter_context(
            tc.tile_pool(name="psG", bufs=1 if with_bias else 2, space="PSUM")
        )

        # Resident weights, loaded once (~4.2MB bf16). wg is DMA'd HERE,
        # before fetch_gate(0) is emitted: tile deps follow emission order,
        # so the gate matmul must be emitted after the wg write.
        wg_sb = consts.tile([128, NDC, E], F32)
        nc.scalar.dma_start(wg_sb[:], wgt.rearrange("(dc p) n -> p dc n", p=128))
        w1_sb = consts.tile([128, NDC, E * DE], BF16)
        w1r = w1t.rearrange("(dc p) n -> p dc n", p=128)
        w2_sb = consts.tile([128, E, D], BF16)
        w2r = w2t.rearrange("(e p) n -> p e n", p=128)
        ident = consts.tile([128, 128], F32)
        make_identity(nc, ident[:])
        identb = consts.tile([128, 128], BF16)
        nc.vector.tensor_copy(identb[:], ident[:])
        if with_bias:
            ones_row = consts.tile([1, 128], F32)
            nc.vector.memset(ones_row[:], 1.0)
            b1_sb = consts.tile([1, E, DE], F32)
            nc.scalar.dma_start(b1_sb[:], b1d[:])
            b2_sb = consts.tile([E, D], F32)
            nc.scalar.dma_start(b2_sb[:], b2d[:])

        def fetch_gate(tt):
            """Emit x DMA + f32r copy + gate-logit MMs + the serial DVE/ACT
            top-2 chain for tile tt.  Called one tile AHEAD so the ~1.5us
            serial gate chain overlaps the previous tile's mm2/mm1 instead
            of stalling the PE before the transposes."""
            tsl = slice(tt * 128, (tt + 1) * 128)
            xt32 = xpool.tile([128, NDC, 128], F32, tag="xt32")
            xr = xT[:, tsl].rearrange("(dc p) t -> p dc t", p=128).bitcast(F32)
            if tt == 0:
                # split tile 0's x into halves: the first gate/mm1 matmuls
                # start on dc 0-3 while dc 4-7 is still in flight
                nc.sync.dma_start(xt32[:, 0:4, :], xr[:, 0:4, :])
                nc.sync.dma_start(xt32[:, 4:8, :], xr[:, 4:8, :])
            else:
                nc.sync.dma_start(xt32[:], xr)
            # bf16 view for the expert matmuls (gate stays exact fp32);
            # cast on ACT: the DVE FIFO carries the gate chain + hg mults
            xt = xpool.tile([128, NDC, 128], BF16, tag="xt")
            nc.scalar.copy(xt[:], xt32[:])
            lg = psG.tile([128, 8], F32, tag="lg")
            for dc in range(NDC):
                nc.tensor.matmul(
                    lg[:], xt32[:, dc, :], wg_sb[:, dc, :],
                    start=dc == 0, stop=dc == NDC - 1,
                )
            # ---- gate: top-2 of 8, weights w1=sigmoid(l1-l2), w2=1-w1
            lg_sb = sbuf.tile([128, 8], F32, tag="lg_sb")
            nc.scalar.copy(lg_sb[:], lg[:])
            mx = sbuf.tile([128, 8], F32, tag="mx")
            nc.vector.max(out=mx[:], in_=lg_sb[:])
            pp = sbuf.tile([128, 3], F32, tag="pp")  # [l1-l2, pa, pb]
            nc.vector.tensor_sub(pp[:, 0:1], mx[:, 0:1], mx[:, 1:2])
            nc.scalar.activation(
                pp[:, 1:2], pp[:, 0:1], mybir.ActivationFunctionType.Sigmoid
            )
            nc.vector.tensor_scalar(
                pp[:, 2:3], pp[:, 1:2], -1.0, 1.0,
                op0=mybir.AluOpType.mult, op1=mybir.AluOpType.add,
            )
            eq = sbuf.tile([128, 2, 8], F32, tag="eq")
            nc.vector.tensor_tensor(
                eq[:, 0, :], lg_sb[:], mx[:, 0:1].to_broadcast([128, 8]),
                mybir.AluOpType.is_equal,
            )
            nc.vector.tensor_tensor(
                eq[:, 1, :], lg_sb[:], mx[:, 1:2].to_broadcast([128, 8]),
                mybir.AluOpType.is_equal,
            )
            nc.vector.tensor_tensor(
                eq[:, 0, :], eq[:, 0, :], pp[:, 1:2].to_broadcast([128, 8]),
                mybir.AluOpType.mult,
            )
            nc.vector.tensor_tensor(
                eq[:, 1, :], eq[:, 1, :], pp[:, 2:3].to_broadcast([128, 8]),
                mybir.AluOpType.mult,
            )
            gate = sbuf.tile([128, 8], F32, tag="gate")
            nc.vector.tensor_add(gate[:], eq[:, 0, :], eq[:, 1, :])
            return xt, gate

        def back_half(hg, gate, tsl, last=False):
            """Transpose hg -> hgT, mm2, and y writeback for one tile.
            last=True: finish all y_lo matmuls first so its writeback
            overlaps the y_hi matmuls (shorter kernel tail)."""
            hgT = sbuf.tile([128, E * 128], BF16, tag="hgT")
            for half in range(2):
                tp = psT.tile([128, 512], BF16, tag="tp")
                for i in range(4):
                    e = half * 4 + i
                    nc.tensor.transpose(
                        tp[:, i * 128:(i + 1) * 128],
                        hg[:, e, :], identb[:],
                    )
                nc.vector.tensor_copy(
                    hgT[:, half * 512:(half + 1) * 512], tp[:]
                )

            # ---- mm2: y[t, d] = sum_e hgT_e.T @ W2T_e  (+ gate @ b2)
            y_lo = psY.tile([128, 512], F32, tag="ylo")
            y_hi = psY.tile([128, 512], F32, tag="yhi")
            halves = ([(0, y_lo)], [(1, y_hi)]) if last else \
                ([(0, y_lo), (1, y_hi)],)
            # y ships as bf16: halves the output HBM traffic; the cast adds
            # ~0.4% of |y| on top of a 2e-2 budget
            y_sb = sbuf.tile([128, 1024], BF16, tag="y")
            for group in halves:
                for e in range(E):
                    lhsT2 = hgT[:, e * 128:(e + 1) * 128]
                    for h, yps in group:
                        nc.tensor.matmul(
                            yps[:], lhsT2, w2_sb[:, e, h * 512:(h + 1) * 512],
                            start=e == 0, stop=e == E - 1 and not with_bias,
                        )
                if last and group[0][0] == 0 and not with_bias:
                    # y_lo done: ship it while the y_hi matmuls run
                    nc.scalar.copy(y_sb[:, 0:512], y_lo[:])
                    nc.scalar.dma_start(y[tsl, 0:512], y_sb[:, 0:512])
            if with_bias:
                gtp = psG.tile([128, 128], F32, tag="gtp")
                nc.tensor.transpose(gtp[0:8, 0:128], gate[:], ident[:])
                gT = sbuf.tile([8, 128], F32, tag="gT")
                nc.vector.tensor_copy(gT[:], gtp[0:8, 0:128])
                nc.tensor.matmul(
                    y_lo[:], gT[:], b2_sb[:, 0:512], start=False, stop=True
                )
                nc.tensor.matmul(
                    y_hi[:], gT[:], b2_sb[:, 512:1024], start=False, stop=True
                )

            # y halves go out as soon as each PSUM->SBUF copy lands, on
            # the scalar DMA queue (x-tile DMAs own the sync queue)
            if not (last and not with_bias):
                nc.scalar.copy(y_sb[:, 0:512], y_lo[:])
                nc.scalar.dma_start(y[tsl, 0:512], y_sb[:, 0:512])
            nc.vector.tensor_copy(y_sb[:, 512:1024], y_hi[:])
            nc.scalar.dma_start(y[tsl, 512:1024], y_sb[:, 512:1024])

        # tile 0's x DMA + gate chain go out BEFORE the 6.3MB of expert
        # weights so the first tile isn't queued behind them on the DMA
        # engines; the per-dc/per-e weight slices then land progressively
        # just ahead of the matmuls that consume them.
        pending = [fetch_gate(0)]
        # interleave w1/w2 chunks 1:1 after a 2-chunk w1 head start, so
        # tile 0's mm2 isn't starved behind the whole w1 stream
        w_order = []
        for k in range(NDC):
            w_order.append(("w1", k))
            if k >= 2:
                w_order.append(("w2", k - 2))
        w_order += [("w2", e) for e in range(NDC - 2, E)]
        for kind, k in w_order:
            if kind == "w1":
                nc.scalar.dma_start(w1_sb[:, k, :], w1r[:, k, :])
            else:
                nc.scalar.dma_start(w2_sb[:, k, :], w2r[:, k, :])

        for _rep in range(reps):
          queue = pending if pending is not None else [fetch_gate(0)]
          pending = None
          prev = None
          for tt in range(NTT):
              tsl = slice(tt * 128, (tt + 1) * 128)
              xt, gate = queue.pop(0)

              # ---- mm1: h[t, (e,de)], accumulate over 8 d-chunks
              h_lo = psA.tile([128, 512], F32, tag="hlo")   # experts 0..3
              h_hi = psA.tile([128, 512], F32, tag="hhi")   # experts 4..7
              for dc in range(NDC):
                  lhsT = xt[:, dc, :]
                  first = dc == 0
                  last = dc == NDC - 1
                  nc.tensor.matmul(
                      h_lo[:], lhsT, w1_sb[:, dc, 0:512],
                      start=first, stop=last and not with_bias,
                  )
                  nc.tensor.matmul(
                      h_hi[:], lhsT, w1_sb[:, dc, 512:1024],
                      start=first, stop=last and not with_bias,
                  )
              if with_bias:
                  # h += b1 via rank-1 matmul: ones[1,128].T @ b1_e[1,128]
                  for e in range(E):
                      tgt = h_lo if e < 4 else h_hi
                      nc.tensor.matmul(
                          tgt[:, (e % 4) * DE:(e % 4 + 1) * DE],
                          ones_row[:], b1_sb[:, e, :],
                          start=False, stop=True,
                      )

              # prefetch + gate for the NEXT tile (overlaps this tile's PE)
              if tt + 1 < NTT:
                  queue.append(fetch_gate(tt + 1))

              # ---- hg = relu(h) * g_e: ACT relu from PSUM, DVE bcast-mult
              hrelu = sbuf.tile([128, E * DE], F32, tag="hrelu")
              hg = sbuf.tile([128, E, DE], BF16, tag="hg")
              for half, hps in ((0, h_lo), (1, h_hi)):
                  nc.scalar.activation(
                      hrelu[:, half * 512:(half + 1) * 512], hps[:],
                      mybir.ActivationFunctionType.Relu,
                  )
                  nc.vector.tensor_tensor(
                      hg[:, half * 4:(half + 1) * 4, :],
                      hrelu.rearrange("p (e d) -> p e d", e=E)[
                          :, half * 4:(half + 1) * 4, :
                      ],
                      gate[:, half * 4:(half + 1) * 4, None].to_broadcast(
                          [128, 4, DE]
                      ),
                      mybir.AluOpType.mult,
                  )

              # back half of the PREVIOUS tile: its hg has been ready for a
              # whole tile, so the transposes never stall the PE waiting on
              # the relu->gate-mult chain of the current tile.
              if prev is not None:
                  back_half(*prev)
              prev = (hg, gate, tsl)
          back_half(*prev, last=True)

    nc.finalize()
    return nc


def _get_nc(with_bias: bool, reps: int = 1):
    key = (with_bias, reps)
    if key not in _NC_CACHE:
        _NC_CACHE[key] = build_moe_nc(with_bias, reps)
    return _NC_CACHE[key]


def _prep_in_maps(inputs, with_bias):
    """Host-side layout prep (pure transposes + sharding, no model math)."""
    x = np.asarray(inputs["x"], np.float32)
    Wg = np.asarray(inputs["Wg"], np.float32)
    W1 = np.asarray(inputs["W1"], np.float32)
    b1 = np.asarray(inputs["b1"], np.float32)
    W2 = np.asarray(inputs["W2"], np.float32)
    b2 = np.asarray(inputs["b2"], np.float32)

    import ml_dtypes

    x2d = x.reshape(T, D)
    xT = np.ascontiguousarray(x2d.T)                                  # [D, T]
    w1t = np.ascontiguousarray(
        np.transpose(W1, (2, 0, 1)).reshape(D, E * DE)
    ).astype(ml_dtypes.bfloat16)
    wgt = np.ascontiguousarray(Wg.T)                                  # [D, E]
    w2t = np.ascontiguousarray(
        np.transpose(W2, (0, 2, 1)).reshape(E * DE, D)
    ).astype(ml_dtypes.bfloat16)

    in_maps = []
    for i in range(NCORES):
        m = {
            "xT": np.ascontiguousarray(xT[:, i * TC:(i + 1) * TC]),
            "w1t": w1t,
            "wgt": wgt,
            "w2t": w2t,
        }
        if with_bias:
            m["b1"] = b1.reshape(1, E, DE).copy()
            m["b2"] = b2.copy()
        in_maps.append(m)
    return in_maps


def kernel(x, Wg, W1, b1, W2, b2):
    global _LAST_RESULT
    inputs = {"x": x, "Wg": Wg, "W1": W1, "b1": b1, "W2": W2, "b2": b2}
    with_bias = bool(np.any(np.asarray(b1))) or bool(np.any(np.asarray(b2)))
    nc = _get_nc(with_bias)
    in_maps = _prep_in_maps(inputs, with_bias)

    trace = bool(int(os.environ.get("MOE_TRACE", "0")))
    res = run_bass_kernel_spmd(nc, in_maps, list(range(NCORES)), trace=trace)
    _LAST_RESULT = res

    y2d = np.concatenate(
        [np.asarray(res.results[i]["y"], np.float32) for i in range(NCORES)],
        axis=0,
    )
    return y2d.reshape(B, S, D)

